# revision 3
# baseline (speedup 1.0000x reference)
"""Center-of-mass pairwise translation + trilinear resample, 8-core Trainium2.

Full inputs x, y: (1,1,192,192,192) f32. Returns (transformed, grid):
  t = com(x) - com(y)                    (3 scalars, normalized coords)
  grid = base_grid + t                   (1,192,192,192,3)
  transformed = trilinear_sample(x, grid) (1,1,192,192,192)

Distribution: shard D across 8 cores (24 planes each, +1 halo plane via
host-side zero-padded shards). Per-core partial center-of-mass sums are
AllReduce'd (8 scalars). The constant translation makes the resample a
separable 3-tap stencil per axis: W on DVE (free-dim shift), H on the
tensor engine (banded 98x96 matmul), D on DVE (section-shifted views).
"""

import numpy as np

import concourse.bass as bass
import concourse.mybir as mybir
from concourse.tile import TileContext
from concourse.bass_utils import run_bass_kernel_spmd

N_CORES = 8
D = H = W = 192
DP = D // N_CORES  # 24 planes per core
F32 = mybir.dt.float32
AL = mybir.AluOpType
AF = mybir.ActivationFunctionType

_CACHE = {}


def _split_excess_waits(nc, max_waits=1):
    """The walrus build in this container rejects instructions carrying more
    than a couple of attached sync waits (Tile's tail drain gets many).
    Hoist the excess onto standalone event-semaphore waits just before."""
    k = 0
    for f in nc.m.functions:
        for bb in f.blocks:
            new_list = []
            changed = False
            for ins in bb.instructions:
                si = getattr(ins, "sync_info", None)
                if si is not None and si.on_wait and len(si.on_wait) > max_waits:
                    waits = list(si.on_wait)
                    keep, excess = waits[:max_waits], waits[max_waits:]
                    for w in excess:
                        k += 1
                        ev = mybir.InstEventSemaphore(
                            name=f"splitw-{k}", ins=[], outs=[]
                        )
                        ev.engine = ins.engine
                        ev.sync_info = mybir.SyncInfo(on_wait=[w], on_update=[])
                        new_list.append(ev)
                    ins.sync_info = mybir.SyncInfo(
                        on_wait=keep, on_update=list(si.on_update)
                    )
                    changed = True
                new_list.append(ins)
            if changed:
                bb.instructions = new_list
    return k


def _build_nc():
    nc = bass.Bass()
    dp = nc.declare_dram_parameter

    xp = dp("xp", [DP + 2, H + 2, W + 2], F32, isOutput=False)  # padded slab
    xf = dp("xf", [DP * H, W], F32, isOutput=False)  # flat slab (stats)
    yf = dp("yf", [DP * H, W], F32, isOutput=False)
    xsf = dp("xsf", [128, W], F32, isOutput=False)  # xs replicated rows
    ys36 = dp("ys36", [128, 36], F32, isOutput=False)  # ys per flat (p,n)
    zs36 = dp("zs36", [128, 36], F32, isOutput=False)  # zs per flat (p,n), per-core
    ma = dp("ma", [128, 96], F32, isOutput=False)  # By diag masks
    mb = dp("mb", [128, 96], F32, isOutput=False)
    mc = dp("mc", [128, 96], F32, isOutput=False)
    xyb0 = dp("xyb0", [128, 3 * W], F32, isOutput=False)  # (xs, ys, 0) chunk templates
    xyb1 = dp("xyb1", [128, 3 * W], F32, isOutput=False)
    zsd = dp("zsd", [128, DP], F32, isOutput=False)  # zs per local plane, per-core
    ones = dp("ones", [128, 1], F32, isOutput=False)
    out_t = dp("out", [DP, H, W], F32, isOutput=True)
    out_g = dp("grid", [DP, H, W, 3], F32, isOutput=True)

    NS = 36  # flat n-sections
    with TileContext(nc) as tc:
        with (
            tc.tile_pool(name="const", bufs=1) as cpool,
            tc.tile_pool(name="big", bufs=2) as big,
            tc.tile_pool(name="vol", bufs=2) as vol,
            tc.tile_pool(name="ot", bufs=2) as otp,
            tc.tile_pool(name="xyt", bufs=8) as xytp,
            tc.tile_pool(name="small", bufs=1) as sm,
            tc.tile_pool(name="psum", bufs=4, space="PSUM") as psp,
            tc.tile_pool(name="psmall", bufs=1, space="PSUM") as pss,
            tc.tile_pool(name="dram", bufs=1, space="DRAM") as dram,
        ):
            # ---------------- const loads ----------------
            xsf_t = cpool.tile([128, W], F32, tag="xsf")
            ys36_t = cpool.tile([128, NS], F32, tag="ys36")
            zs36_t = cpool.tile([128, NS], F32, tag="zs36")
            ma_t = cpool.tile([128, 96], F32, tag="ma")
            mb_t = cpool.tile([128, 96], F32, tag="mb")
            mc_t = cpool.tile([128, 96], F32, tag="mc")
            xyb_t = [cpool.tile([128, 3 * W], F32, tag=f"xyb{k}", name=f"xyb_t{k}") for k in (0, 1)]
            zsd_t = cpool.tile([128, DP], F32, tag="zsd")
            ones_t = cpool.tile([128, 1], F32, tag="ones")
            for t, src in [
                (xsf_t, xsf), (ys36_t, ys36), (zs36_t, zs36), (ma_t, ma),
                (mb_t, mb), (mc_t, mc), (xyb_t[0], xyb0), (xyb_t[1], xyb1),
                (zsd_t, zsd), (ones_t, ones),
            ]:
                nc.sync.dma_start(out=t[:], in_=src[:])

            # ---------------- data loads ----------------
            xfb = big.tile([128, NS * W], F32, tag="bigslab")
            yfb = big.tile([128, NS * W], F32, tag="bigslab")
            nc.sync.dma_start(
                out=xfb[:].rearrange("p (n w) -> p n w", n=NS),
                in_=xf[:].rearrange("(n p) w -> p n w", p=128),
            )
            nc.sync.dma_start(
                out=yfb[:].rearrange("p (n w) -> p n w", n=NS),
                in_=yf[:].rearrange("(n p) w -> p n w", p=128),
            )
            # padded tiles: partitions = h rows [h0, h0+98), free = (d, w)
            NDP = DP + 2
            xts = []
            for k, h0 in enumerate((0, 96)):
                xt = vol.tile([128, NDP * (W + 2)], F32, tag="volslab", name=f"xt{k}")
                nc.sync.dma_start(
                    out=xt[0:98, :].rearrange("p (d w) -> p d w", d=NDP),
                    in_=xp[:, h0 : h0 + 98, :].transpose([1, 0, 2]),
                )
                xts.append(xt)

            # ---------------- stats ----------------
            st = sm.tile([128, 8], F32, tag="st")
            rs = [sm.tile([128, NS], F32, tag=f"rs{i}", name=f"rs{i}") for i in range(2)]
            scr = sm.tile([128, NS], F32, tag="scr")
            xsb3 = xsf_t[:, :].unsqueeze(1).broadcast_to([128, NS, W])
            for i, tile in enumerate((xfb, yfb)):
                c0 = 4 * i
                v3 = tile[:, :].rearrange("p (n w) -> p n w", n=NS)
                nc.vector.tensor_reduce(
                    rs[i][:, :], v3, axis=mybir.AxisListType.X, op=AL.add
                )
                nc.vector.tensor_reduce(
                    st[:, c0 : c0 + 1], rs[i][:, :], axis=mybir.AxisListType.X,
                    op=AL.add,
                )
                # mw: sum(x * xs[w]) — in-place, accumulate over all free dims
                nc.vector.scalar_tensor_tensor(
                    out=v3, in0=v3, scalar=1.0, in1=xsb3, op0=AL.mult, op1=AL.mult,
                    accum_out=st[:, c0 + 1 : c0 + 2],
                )
                # mh: sum(rowsum * ys[h])
                nc.vector.scalar_tensor_tensor(
                    out=scr[:, :], in0=rs[i][:, :], scalar=1.0, in1=ys36_t[:, :],
                    op0=AL.mult, op1=AL.mult, accum_out=st[:, c0 + 2 : c0 + 3],
                )
                # md: sum(rowsum * zs[d])
                nc.vector.scalar_tensor_tensor(
                    out=scr[:, :], in0=rs[i][:, :], scalar=1.0, in1=zs36_t[:, :],
                    op0=AL.mult, op1=AL.mult, accum_out=st[:, c0 + 3 : c0 + 4],
                )

            # reduce partials over partitions: [8,1] = st.T @ ones
            pst = pss.tile([8, 1], F32, tag="pst")
            nc.tensor.matmul(pst[:, :], st[:, :], ones_t[:, :], start=True, stop=True)

            sts = sm.tile([128, 1], F32, tag="sts")
            nc.scalar.copy(sts[0:8, 0:1], pst[:, :])
            cc_in = dram.tile([1, 8], F32)
            cc_out = dram.tile([1, 8], F32)
            nc.sync.dma_start(out=cc_in[0:1, 0:8], in_=sts[0:8, 0:1])
            nc.gpsimd.collective_compute(
                "AllReduce", AL.add,
                replica_groups=[list(range(N_CORES))],
                ins=[cc_in[:].opt()], outs=[cc_out[:].opt()],
            )
            s8 = sm.tile([128, 8], F32, tag="s8")
            nc.sync.dma_start(out=s8[:, :], in_=cc_out[0:1, 0:8].broadcast_to([128, 8]))

            # ---------------- translation + stencil weights ----------------
            w3 = sm.tile([128, 24], F32, tag="w3")  # scratch columns
            RX, RY = w3[:, 0:1], w3[:, 1:2]
            CMX, CMY = w3[:, 2:5], w3[:, 5:8]
            T3, O3 = w3[:, 8:11], w3[:, 11:14]
            A3, C3 = w3[:, 14:17], w3[:, 17:20]
            B3 = w3[:, 20:23]
            w4 = sm.tile([128, 16], F32, tag="w4")
            RB3, AR3, CR3 = w4[:, 0:3], w4[:, 3:6], w4[:, 6:9]
            BP, AP_, CP = w4[:, 9:10], w4[:, 10:11], w4[:, 11:12]
            NEG3 = w4[:, 12:15]

            nc.vector.reciprocal(RX, s8[:, 0:1])
            nc.vector.reciprocal(RY, s8[:, 4:5])
            nc.vector.tensor_scalar_mul(CMX, s8[:, 1:4], RX)
            nc.vector.tensor_scalar_mul(CMY, s8[:, 5:8], RY)
            nc.vector.tensor_tensor(out=T3, in0=CMX, in1=CMY, op=AL.subtract)
            nc.vector.tensor_scalar_mul(O3, T3, (W - 1) / 2.0)
            nc.vector.tensor_scalar_mul(NEG3, O3, -1.0)
            nc.vector.tensor_scalar_max(A3, NEG3, 0.0)
            nc.vector.tensor_scalar_max(C3, O3, 0.0)
            nc.vector.tensor_tensor(out=B3, in0=A3, in1=C3, op=AL.add)
            nc.vector.tensor_scalar(B3, B3, -1.0, 1.0, AL.mult, AL.add)  # 1-(a+c)
            nc.vector.reciprocal(RB3, B3)
            nc.vector.tensor_tensor(out=AR3, in0=A3, in1=RB3, op=AL.mult)
            nc.vector.tensor_tensor(out=CR3, in0=C3, in1=RB3, op=AL.mult)
            nc.vector.tensor_tensor(out=BP, in0=B3[:, 0:1], in1=B3[:, 2:3], op=AL.mult)
            nc.vector.tensor_tensor(out=AP_, in0=A3[:, 2:3], in1=B3[:, 0:1], op=AL.mult)
            nc.vector.tensor_tensor(out=CP, in0=C3[:, 2:3], in1=B3[:, 0:1], op=AL.mult)

            zt = sm.tile([128, DP], F32, tag="zt")
            nc.vector.tensor_scalar_add(zt[:, :], zsd_t[:, :], T3[:, 2:3])

            # By (h-axis 3-tap as a banded [98,96] matrix), raw (ay,by,cy)
            by_t = sm.tile([128, 96], F32, tag="by")
            nc.vector.tensor_scalar_mul(by_t[:, :], ma_t[:, :], A3[:, 1:2])
            nc.vector.scalar_tensor_tensor(
                out=by_t[:, :], in0=mb_t[:, :], scalar=B3[:, 1:2], in1=by_t[:, :],
                op0=AL.mult, op1=AL.add,
            )
            nc.vector.scalar_tensor_tensor(
                out=by_t[:, :], in0=mc_t[:, :], scalar=C3[:, 1:2], in1=by_t[:, :],
                op0=AL.mult, op1=AL.add,
            )

            # ---------------- resample + grid, per h-chunk ----------------
            NB = 13  # 2-plane matmul batches over 26 padded planes
            for k, h0 in enumerate((0, 96)):
                xt = xts[k]
                x3 = xt[0:98, :].rearrange("p (d w) -> p d w", d=NDP)
                sxt = big.tile([128, NDP * W], F32, tag="bigslab", name=f"sxt{k}")
                s3 = sxt[0:98, :].rearrange("p (d w) -> p d w", d=NDP)
                # W-axis 3-tap (ratio weights, scale deferred)
                nc.vector.scalar_tensor_tensor(
                    out=s3, in0=x3[:, :, 0:W], scalar=AR3[0:98, 0:1],
                    in1=x3[:, :, 1 : W + 1], op0=AL.mult, op1=AL.add,
                )
                nc.vector.scalar_tensor_tensor(
                    out=s3, in0=x3[:, :, 2 : W + 2], scalar=CR3[0:98, 0:1],
                    in1=s3, op0=AL.mult, op1=AL.add,
                )
                # H-axis via PE: [96, 2W] = by.T @ sxt-batch
                sy = vol.tile([128, NDP * W], F32, tag="volslab", name=f"sy{k}")
                for b in range(NB):
                    fr = slice(b * 2 * W, (b + 1) * 2 * W)
                    ps = psp.tile([96, 2 * W], F32, tag="syp", name=f"ps{k}_{b}")
                    nc.tensor.matmul(
                        ps[:, :], by_t[0:98, :], sxt[0:98, fr], start=True, stop=True
                    )
                    nc.scalar.copy(sy[0:96, fr], ps[:, :])
                # D-axis 3-tap across sections + total scale
                y3 = sy[0:96, :].rearrange("p (d w) -> p d w", d=NDP)
                ot = otp.tile([128, DP * W], F32, tag="ot", name=f"ot{k}")
                o3 = ot[0:96, :].rearrange("p (d w) -> p d w", d=DP)
                nc.vector.tensor_scalar_mul(o3, y3[:, 1 : DP + 1, :], BP[0:96, 0:1])
                nc.vector.scalar_tensor_tensor(
                    out=o3, in0=y3[:, 0:DP, :], scalar=AP_[0:96, 0:1], in1=o3,
                    op0=AL.mult, op1=AL.add,
                )
                nc.vector.scalar_tensor_tensor(
                    out=o3, in0=y3[:, 2 : DP + 2, :], scalar=CP[0:96, 0:1], in1=o3,
                    op0=AL.mult, op1=AL.add,
                )
                nc.sync.dma_start(
                    out=out_t[:, h0 : h0 + 96, :].transpose([1, 0, 2]), in_=o3
                )

                # grid: plane template (xs+tx, ys+ty, z-filled per plane)
                bufs = [xytp.tile([128, 3 * W], F32, tag="xytbuf", name=f"xyt{k}_{i}") for i in range(4)]
                m = bufs[0]
                nc.vector.tensor_copy(m[:, :], xyb_t[k][:, :])
                m3 = m[:, :].rearrange("p (w c) -> p w c", c=3)
                nc.vector.tensor_scalar_add(m3[:, :, 0:1], m3[:, :, 0:1], T3[:, 0:1])
                nc.vector.tensor_scalar_add(m3[:, :, 1:2], m3[:, :, 1:2], T3[:, 1:2])
                for b in bufs[1:]:
                    nc.vector.tensor_copy(b[:, :], m[:, :])
                for d in range(DP):
                    b = bufs[d % 4]
                    zcol = b[0:96, :].rearrange("p (w c) -> p w c", c=3)[:, :, 2:3]
                    nc.scalar.activation(
                        zcol, zcol, AF.Identity, bias=zt[0:96, d : d + 1], scale=0.0
                    )
                    nc.sync.dma_start(
                        out=out_g[d, h0 : h0 + 96].rearrange("h w c -> h (w c)"),
                        in_=b[0:96, :],
                    )

    _split_excess_waits(nc)
    return nc


def _shards(x3, y3):
    xs = np.linspace(-1.0, 1.0, W, dtype=np.float64).astype(np.float32)
    ys = np.linspace(-1.0, 1.0, H, dtype=np.float64).astype(np.float32)
    zs = np.linspace(-1.0, 1.0, D, dtype=np.float64).astype(np.float32)

    xpad = np.zeros((D + 2, H + 2, W + 2), dtype=np.float32)
    xpad[1:-1, 1:-1, 1:-1] = x3

    NS = 36
    p_idx = np.arange(128)
    n_idx = np.arange(NS)
    rows = n_idx[None, :] * 128 + p_idx[:, None]  # [128, NS]
    hh = rows % H
    dd = rows // H
    ys36 = ys[hh]

    ma = np.zeros((128, 96), np.float32)
    mb = np.zeros((128, 96), np.float32)
    mc = np.zeros((128, 96), np.float32)
    j = np.arange(96)
    ma[j, j] = 1.0
    mb[j + 1, j] = 1.0
    mc[j + 2, j] = 1.0

    xyb = []
    for h0 in (0, 96):
        t = np.zeros((128, 3 * W), np.float32)
        t[0:96, 0::3] = xs[None, :]
        t[0:96, 1::3] = ys[h0 : h0 + 96, None]
        xyb.append(t)

    ones = np.ones((128, 1), np.float32)

    in_maps = []
    for c in range(N_CORES):
        d0 = DP * c
        zloc = zs[d0 + dd]  # [128, NS] — zs at global d of each flat row
        zsd = np.broadcast_to(zs[d0 : d0 + DP][None, :], (128, DP)).copy()
        in_maps.append({
            "xp": np.ascontiguousarray(xpad[d0 : d0 + DP + 2]),
            "xf": np.ascontiguousarray(x3[d0 : d0 + DP].reshape(DP * H, W)),
            "yf": np.ascontiguousarray(y3[d0 : d0 + DP].reshape(DP * H, W)),
            "xsf": np.broadcast_to(xs[None, :], (128, W)).copy(),
            "ys36": np.ascontiguousarray(ys36.astype(np.float32)),
            "zs36": np.ascontiguousarray(zloc.astype(np.float32)),
            "ma": ma, "mb": mb, "mc": mc,
            "xyb0": xyb[0], "xyb1": xyb[1],
            "zsd": zsd, "ones": ones,
        })
    return in_maps


def kernel(x, y):
    x = np.asarray(x, dtype=np.float32)
    y = np.asarray(y, dtype=np.float32)
    x3 = x.reshape(D, H, W)
    y3 = y.reshape(D, H, W)

    if "nc" not in _CACHE:
        _CACHE["nc"] = _build_nc()
    nc = _CACHE["nc"]

    in_maps = _shards(x3, y3)
    res = run_bass_kernel_spmd(nc, in_maps, list(range(N_CORES)))

    transformed = np.empty((1, 1, D, H, W), np.float32)
    grid = np.empty((1, D, H, W, 3), np.float32)
    for c in range(N_CORES):
        transformed[0, 0, DP * c : DP * (c + 1)] = res.results[c]["out"]
        grid[0, DP * c : DP * (c + 1)] = res.results[c]["grid"]
    return transformed, grid


# revision 7
# speedup vs baseline: 1.2198x; 1.2198x over previous
"""Center-of-mass pairwise translation + trilinear resample, 8-core Trainium2.

Full inputs x, y: (1,1,192,192,192) f32. Returns (transformed, grid):
  t = com(x) - com(y)                    (3 scalars, normalized coords)
  grid = base_grid + t                   (1,192,192,192,3)
  transformed = trilinear_sample(x, grid) (1,1,192,192,192)

Distribution: shard D across 8 cores (24 planes each, +1 halo plane via
host-side zero-padded shards). Per-core partial center-of-mass sums are
AllGather'd (32 scalars/core) and combined locally. The constant translation
makes the resample a separable 3-tap stencil per axis: W on DVE (free-dim
shift), H on the tensor engine (banded 98x96 matmul), D on DVE
(section-shifted views). The grid output is generated on gpsimd (plane
templates + per-plane z fill) while DVE does the resample.
"""

import numpy as np

import concourse.bass as bass
import concourse.mybir as mybir
from concourse.tile import TileContext
from concourse.bass_utils import run_bass_kernel_spmd

N_CORES = 8
D = H = W = 192
DP = D // N_CORES  # 24 planes per core
F32 = mybir.dt.float32
AL = mybir.AluOpType
AF = mybir.ActivationFunctionType

_CACHE = {}


def _split_excess_waits(nc, max_waits=1):
    """The walrus build in this container rejects instructions carrying more
    than a couple of attached sync waits (Tile's tail drain gets many).
    Hoist the excess onto standalone event-semaphore waits just before."""
    k = 0
    for f in nc.m.functions:
        for bb in f.blocks:
            new_list = []
            changed = False
            for ins in bb.instructions:
                si = getattr(ins, "sync_info", None)
                if si is not None and si.on_wait and len(si.on_wait) > max_waits:
                    waits = list(si.on_wait)
                    keep, excess = waits[:max_waits], waits[max_waits:]
                    for w in excess:
                        k += 1
                        ev = mybir.InstEventSemaphore(
                            name=f"splitw-{k}", ins=[], outs=[]
                        )
                        ev.engine = ins.engine
                        ev.sync_info = mybir.SyncInfo(on_wait=[w], on_update=[])
                        new_list.append(ev)
                    ins.sync_info = mybir.SyncInfo(
                        on_wait=keep, on_update=list(si.on_update)
                    )
                    changed = True
                new_list.append(ins)
            if changed:
                bb.instructions = new_list
    return k


NS = 36  # flat n-sections (rows of 128 partitions)
NQ = NS // 4  # 9 sections per stats quarter
NDP = DP + 2  # padded d sections
HS = NDP // 2 + 1  # 14: d-sections per resample half (1-section overlap)


def _build_nc():
    nc = bass.Bass()
    dp = nc.declare_dram_parameter

    xp = dp("xp", [NDP, H + 2, W + 2], F32, isOutput=False)  # padded slab
    xf = dp("xf", [DP * H, W], F32, isOutput=False)  # flat slab (stats)
    yf = dp("yf", [DP * H, W], F32, isOutput=False)
    xsf = dp("xsf", [128, W], F32, isOutput=False)  # xs replicated rows
    ys36 = dp("ys36", [128, NS], F32, isOutput=False)  # ys per flat (p,n)
    zs36 = dp("zs36", [128, NS], F32, isOutput=False)  # zs per flat (p,n), per-core
    ma = dp("ma", [128, 96], F32, isOutput=False)  # By diag masks
    mb = dp("mb", [128, 96], F32, isOutput=False)
    mc = dp("mc", [128, 96], F32, isOutput=False)
    xyb0 = dp("xyb0", [128, 3 * W], F32, isOutput=False)  # (xs, ys, 0) templates
    xyb1 = dp("xyb1", [128, 3 * W], F32, isOutput=False)
    zsd = dp("zsd", [128, DP], F32, isOutput=False)  # zs per local plane, per-core
    ones = dp("ones", [128, 1], F32, isOutput=False)
    out_t = dp("out", [DP, H, W], F32, isOutput=True)
    out_g = dp("grid", [DP, H, W, 3], F32, isOutput=True)

    with TileContext(nc) as tc:
        with (
            tc.tile_pool(name="const", bufs=1) as cpool,
            tc.tile_pool(name="big", bufs=5) as big,
            tc.tile_pool(name="vol", bufs=4) as vol,
            tc.tile_pool(name="ot", bufs=2) as otp,
            tc.tile_pool(name="xyt", bufs=8) as xytp,
            tc.tile_pool(name="small", bufs=1) as sm,
            tc.tile_pool(name="psum", bufs=6, space="PSUM") as psp,
            tc.tile_pool(name="psmall", bufs=1, space="PSUM") as pss,
            tc.tile_pool(name="dram", bufs=1, space="DRAM") as dram,
        ):
            # ---------------- const loads ----------------
            xsf_t = cpool.tile([128, W], F32, tag="xsf")
            ys36_t = cpool.tile([128, NS], F32, tag="ys36")
            zs36_t = cpool.tile([128, NS], F32, tag="zs36")
            ma_t = cpool.tile([128, 96], F32, tag="ma")
            mb_t = cpool.tile([128, 96], F32, tag="mb")
            mc_t = cpool.tile([128, 96], F32, tag="mc")
            xyb_t = [
                cpool.tile([128, 3 * W], F32, tag=f"xyb{k}", name=f"xyb_t{k}")
                for k in (0, 1)
            ]
            zsd_t = cpool.tile([128, DP], F32, tag="zsd")
            ones_t = cpool.tile([128, 1], F32, tag="ones")
            for t, src in [
                (xsf_t, xsf), (ys36_t, ys36), (zs36_t, zs36), (ma_t, ma),
                (mb_t, mb), (mc_t, mc), (xyb_t[0], xyb0), (xyb_t[1], xyb1),
                (zsd_t, zsd), (ones_t, ones),
            ]:
                nc.sync.dma_start(out=t[:], in_=src[:])

            # ---------------- stats quarter loads (x/y interleaved) --------
            quarters = []  # (tile, t_idx, q_idx)
            for q in range(4):
                for ti, src in enumerate((xf, yf)):
                    qt = big.tile(
                        [128, NQ * W], F32, tag="big", name=f"q_{ti}_{q}"
                    )
                    nc.sync.dma_start(
                        out=qt[:].rearrange("p (n w) -> p n w", n=NQ),
                        in_=src[:]
                        .rearrange("(n p) w -> p n w", p=128)[
                            :, q * NQ : (q + 1) * NQ, :
                        ],
                    )
                    quarters.append((qt, ti, q))

            # ---------------- stats (per quarter: s, mw, mh, md) -----------
            st = sm.tile([128, 32], F32, tag="st")
            scr = sm.tile([128, NQ], F32, tag="scr")
            rsq = [
                sm.tile([128, NQ], F32, tag=f"rs{i}", name=f"rs{i}")
                for i in range(8)
            ]
            xsb3 = xsf_t[:, :].unsqueeze(1).broadcast_to([128, NQ, W])
            for i, (qt, ti, q) in enumerate(quarters):
                c0 = ti * 16 + q * 4
                nsl = slice(q * NQ, (q + 1) * NQ)
                v3 = qt[:, :].rearrange("p (n w) -> p n w", n=NQ)
                nc.vector.tensor_reduce(
                    rsq[i][:, :], v3, axis=mybir.AxisListType.X, op=AL.add
                )
                nc.vector.tensor_reduce(
                    st[:, c0 : c0 + 1], rsq[i][:, :], axis=mybir.AxisListType.X,
                    op=AL.add,
                )
                # mw: sum(x * xs[w]) — in-place, accumulate over all free dims
                nc.vector.scalar_tensor_tensor(
                    out=v3, in0=v3, scalar=1.0, in1=xsb3, op0=AL.mult,
                    op1=AL.mult, accum_out=st[:, c0 + 1 : c0 + 2],
                )
                # mh: sum(rowsum * ys[h]);  md: sum(rowsum * zs[d])
                nc.vector.scalar_tensor_tensor(
                    out=scr[:, :], in0=rsq[i][:, :], scalar=1.0,
                    in1=ys36_t[:, nsl], op0=AL.mult, op1=AL.mult,
                    accum_out=st[:, c0 + 2 : c0 + 3],
                )
                nc.vector.scalar_tensor_tensor(
                    out=scr[:, :], in0=rsq[i][:, :], scalar=1.0,
                    in1=zs36_t[:, nsl], op0=AL.mult, op1=AL.mult,
                    accum_out=st[:, c0 + 3 : c0 + 4],
                )

            # partition-reduce the 32 partials, AllGather, combine locally
            pst = pss.tile([32, 1], F32, tag="pst")
            nc.tensor.matmul(pst[:, :], st[:, :], ones_t[:, :], start=True, stop=True)
            sts = sm.tile([128, 1], F32, tag="sts")
            nc.scalar.copy(sts[0:32, 0:1], pst[:, :])
            cc_in = dram.tile([1, 32], F32)
            cc_out = dram.tile([1, 32 * N_CORES], F32)
            nc.sync.dma_start(out=cc_in[0:1, 0:32], in_=sts[0:32, 0:1])
            nc.gpsimd.collective_compute(
                "AllGather", AL.bypass,
                replica_groups=[list(range(N_CORES))],
                ins=[cc_in[:].opt()], outs=[cc_out[:].opt()],
            )
            s256 = sm.tile([128, 32 * N_CORES], F32, tag="s256")
            nc.sync.dma_start(
                out=s256[:, :],
                in_=cc_out[0:1, :].broadcast_to([128, 32 * N_CORES]),
            )
            # sum over ranks (stride 32), then over quarters (stride 4)
            a32 = sm.tile([128, 32], F32, tag="a32")
            nc.vector.tensor_reduce(
                a32[:, :],
                s256[:, :].rearrange("p (r c) -> p c r", r=N_CORES),
                axis=mybir.AxisListType.X, op=AL.add,
            )
            s8 = sm.tile([128, 8], F32, tag="s8")
            nc.vector.tensor_reduce(
                s8[:, :].rearrange("p (t s) -> p t s", t=2),
                a32[:, :].rearrange("p (t q s) -> p t s q", t=2, q=4),
                axis=mybir.AxisListType.X, op=AL.add,
            )

            # ------------- grid buffers: copy templates during AllGather ----
            gbufs = {}
            for k in (0, 1):
                gbufs[k] = [
                    xytp.tile([128, 3 * W], F32, tag="xytbuf", name=f"xyt{k}_{i}")
                    for i in range(4)
                ]
                for b in gbufs[k]:
                    nc.vector.tensor_copy(b[:, :], xyb_t[k][:, :])

            # ---------------- padded volume loads (fills DMA slack) --------
            xts = []
            for k, h0 in enumerate((0, 96)):
                xt = vol.tile([128, NDP * (W + 2)], F32, tag="vol", name=f"xt{k}")
                nc.sync.dma_start(
                    out=xt[0:98, :].rearrange("p (d w) -> p d w", d=NDP),
                    in_=xp[:, h0 : h0 + 98, :].transpose([1, 0, 2]),
                )
                xts.append(xt)

            # ---------------- translation + stencil weights ----------------
            w3 = sm.tile([128, 24], F32, tag="w3")
            RX, RY = w3[:, 0:1], w3[:, 1:2]
            CMX, CMY = w3[:, 2:5], w3[:, 5:8]
            T3, O3 = w3[:, 8:11], w3[:, 11:14]
            A3, C3 = w3[:, 14:17], w3[:, 17:20]
            B3 = w3[:, 20:23]
            w4 = sm.tile([128, 16], F32, tag="w4")
            RB3, AR3, CR3 = w4[:, 0:3], w4[:, 3:6], w4[:, 6:9]
            BP, AP_, CP = w4[:, 9:10], w4[:, 10:11], w4[:, 11:12]
            NEG3 = w4[:, 12:15]

            nc.vector.reciprocal(RX, s8[:, 0:1])
            nc.vector.reciprocal(RY, s8[:, 4:5])
            nc.vector.tensor_scalar_mul(CMX, s8[:, 1:4], RX)
            nc.vector.tensor_scalar_mul(CMY, s8[:, 5:8], RY)
            nc.vector.tensor_tensor(out=T3, in0=CMX, in1=CMY, op=AL.subtract)
            nc.vector.tensor_scalar_mul(O3, T3, (W - 1) / 2.0)
            nc.vector.tensor_scalar_mul(NEG3, O3, -1.0)
            nc.vector.tensor_scalar_max(A3, NEG3, 0.0)
            nc.vector.tensor_scalar_max(C3, O3, 0.0)
            nc.vector.tensor_tensor(out=B3, in0=A3, in1=C3, op=AL.add)
            nc.vector.tensor_scalar(B3, B3, -1.0, 1.0, AL.mult, AL.add)  # 1-(a+c)
            nc.vector.reciprocal(RB3, B3)
            nc.vector.tensor_tensor(out=AR3, in0=A3, in1=RB3, op=AL.mult)
            nc.vector.tensor_tensor(out=CR3, in0=C3, in1=RB3, op=AL.mult)
            nc.vector.tensor_tensor(out=BP, in0=B3[:, 0:1], in1=B3[:, 2:3], op=AL.mult)
            nc.vector.tensor_tensor(out=AP_, in0=A3[:, 2:3], in1=B3[:, 0:1], op=AL.mult)
            nc.vector.tensor_tensor(out=CP, in0=C3[:, 2:3], in1=B3[:, 0:1], op=AL.mult)

            zt = sm.tile([128, DP], F32, tag="zt")
            nc.vector.tensor_scalar_add(zt[:, :], zsd_t[:, :], T3[:, 2:3])

            # By (h-axis 3-tap as a banded [98,96] matrix), raw (ay,by,cy)
            by_t = sm.tile([128, 96], F32, tag="by")
            nc.vector.tensor_scalar_mul(by_t[:, :], ma_t[:, :], A3[:, 1:2])
            nc.vector.scalar_tensor_tensor(
                out=by_t[:, :], in0=mb_t[:, :], scalar=B3[:, 1:2], in1=by_t[:, :],
                op0=AL.mult, op1=AL.add,
            )
            nc.vector.scalar_tensor_tensor(
                out=by_t[:, :], in0=mc_t[:, :], scalar=C3[:, 1:2], in1=by_t[:, :],
                op0=AL.mult, op1=AL.add,
            )

            # ---------------- grid generation (gpsimd + sync DMA) ----------
            for k, h0 in enumerate((0, 96)):
                for b in gbufs[k]:
                    b3v = b[:, :].rearrange("p (w c) -> p w c", c=3)
                    nc.gpsimd.tensor_scalar_add(b3v[:, :, 0:1], b3v[:, :, 0:1], T3[:, 0:1])
                    nc.gpsimd.tensor_scalar_add(b3v[:, :, 1:2], b3v[:, :, 1:2], T3[:, 1:2])
                for d in range(DP):
                    b = gbufs[k][d % 4]
                    zcol = b[0:96, :].rearrange("p (w c) -> p w c", c=3)[:, :, 2:3]
                    nc.gpsimd.tensor_scalar(
                        zcol, zcol, 0.0, zt[0:96, d : d + 1], AL.mult, AL.add
                    )
                    nc.sync.dma_start(
                        out=out_g[d, h0 : h0 + 96].rearrange("h w c -> h (w c)"),
                        in_=b[0:96, :],
                    )

            # ---------------- resample: per chunk, per d-half --------------
            for k, h0 in enumerate((0, 96)):
                xt = xts[k]
                x3 = xt[0:98, :].rearrange("p (d w) -> p d w", d=NDP)
                for hf in range(2):
                    s0 = hf * (NDP - HS)  # 0 or 12: padded-section range start
                    xv = x3[:, s0 : s0 + HS, :]
                    sxt = big.tile(
                        [128, HS * W], F32, tag="big", name=f"sxt{k}_{hf}"
                    )
                    s3 = sxt[0:98, :].rearrange("p (d w) -> p d w", d=HS)
                    # W-axis 3-tap (ratio weights, scale deferred)
                    nc.vector.scalar_tensor_tensor(
                        out=s3, in0=xv[:, :, 0:W], scalar=AR3[0:98, 0:1],
                        in1=xv[:, :, 1 : W + 1], op0=AL.mult, op1=AL.add,
                    )
                    nc.vector.scalar_tensor_tensor(
                        out=s3, in0=xv[:, :, 2 : W + 2], scalar=CR3[0:98, 0:1],
                        in1=s3, op0=AL.mult, op1=AL.add,
                    )
                    # H-axis via PE in 512-col batches
                    sy = vol.tile([128, HS * W], F32, tag="vol", name=f"sy{k}_{hf}")
                    nfree = HS * W
                    for b, c in enumerate(range(0, nfree, 512)):
                        n = min(512, nfree - c)
                        ps = psp.tile([96, 512], F32, tag="syp", name=f"ps{k}_{hf}_{b}")
                        nc.tensor.matmul(
                            ps[:, 0:n], by_t[0:98, :], sxt[0:98, c : c + n],
                            start=True, stop=True,
                        )
                        nc.scalar.copy(sy[0:96, c : c + n], ps[:, 0:n])
                    # D-axis 3-tap + total scale; mid-tap scale on ACT
                    HD = HS - 2  # 12 output planes per half
                    y3 = sy[0:96, :].rearrange("p (d w) -> p d w", d=HS)
                    ot = otp.tile([128, HD * W], F32, tag="ot", name=f"ot{k}_{hf}")
                    o3 = ot[0:96, :].rearrange("p (d w) -> p d w", d=HD)
                    nc.scalar.mul(o3, y3[:, 1 : HD + 1, :], BP[0:96, 0:1])
                    nc.vector.scalar_tensor_tensor(
                        out=o3, in0=y3[:, 0:HD, :], scalar=AP_[0:96, 0:1], in1=o3,
                        op0=AL.mult, op1=AL.add,
                    )
                    nc.vector.scalar_tensor_tensor(
                        out=o3, in0=y3[:, 2 : HD + 2, :], scalar=CP[0:96, 0:1],
                        in1=o3, op0=AL.mult, op1=AL.add,
                    )
                    d0 = hf * HD
                    nc.sync.dma_start(
                        out=out_t[d0 : d0 + HD, h0 : h0 + 96, :].transpose([1, 0, 2]),
                        in_=o3,
                    )

    _split_excess_waits(nc)
    return nc


def _shards(x3, y3):
    xs = np.linspace(-1.0, 1.0, W, dtype=np.float64).astype(np.float32)
    ys = np.linspace(-1.0, 1.0, H, dtype=np.float64).astype(np.float32)
    zs = np.linspace(-1.0, 1.0, D, dtype=np.float64).astype(np.float32)

    xpad = np.zeros((D + 2, H + 2, W + 2), dtype=np.float32)
    xpad[1:-1, 1:-1, 1:-1] = x3

    p_idx = np.arange(128)
    n_idx = np.arange(NS)
    rows = n_idx[None, :] * 128 + p_idx[:, None]  # [128, NS]
    hh = rows % H
    dd = rows // H
    ys36 = ys[hh]

    ma = np.zeros((128, 96), np.float32)
    mb = np.zeros((128, 96), np.float32)
    mc = np.zeros((128, 96), np.float32)
    j = np.arange(96)
    ma[j, j] = 1.0
    mb[j + 1, j] = 1.0
    mc[j + 2, j] = 1.0

    xyb = []
    for h0 in (0, 96):
        t = np.zeros((128, 3 * W), np.float32)
        t[0:96, 0::3] = xs[None, :]
        t[0:96, 1::3] = ys[h0 : h0 + 96, None]
        xyb.append(t)

    ones = np.ones((128, 1), np.float32)

    in_maps = []
    for c in range(N_CORES):
        d0 = DP * c
        zloc = zs[d0 + dd]  # [128, NS] — zs at global d of each flat row
        zsd = np.broadcast_to(zs[d0 : d0 + DP][None, :], (128, DP)).copy()
        in_maps.append({
            "xp": np.ascontiguousarray(xpad[d0 : d0 + DP + 2]),
            "xf": np.ascontiguousarray(x3[d0 : d0 + DP].reshape(DP * H, W)),
            "yf": np.ascontiguousarray(y3[d0 : d0 + DP].reshape(DP * H, W)),
            "xsf": np.broadcast_to(xs[None, :], (128, W)).copy(),
            "ys36": np.ascontiguousarray(ys36.astype(np.float32)),
            "zs36": np.ascontiguousarray(zloc.astype(np.float32)),
            "ma": ma, "mb": mb, "mc": mc,
            "xyb0": xyb[0], "xyb1": xyb[1],
            "zsd": zsd, "ones": ones,
        })
    return in_maps


def kernel(x, y):
    x = np.asarray(x, dtype=np.float32)
    y = np.asarray(y, dtype=np.float32)
    x3 = x.reshape(D, H, W)
    y3 = y.reshape(D, H, W)

    if "nc" not in _CACHE:
        _CACHE["nc"] = _build_nc()
    nc = _CACHE["nc"]

    in_maps = _shards(x3, y3)
    res = run_bass_kernel_spmd(nc, in_maps, list(range(N_CORES)))

    transformed = np.empty((1, 1, D, H, W), np.float32)
    grid = np.empty((1, D, H, W, 3), np.float32)
    for c in range(N_CORES):
        transformed[0, 0, DP * c : DP * (c + 1)] = res.results[c]["out"]
        grid[0, DP * c : DP * (c + 1)] = res.results[c]["grid"]
    return transformed, grid
